# revision 5
# baseline (speedup 1.0000x reference)
"""Trainium2 Bass kernel for nn_MultiHeadAttention (B=4, S=2048, D=1024, H=16).

Sharding: tensor-parallel over heads (2 heads per core, 8 cores). Each core:
  1. Projects Q/K (feature-major, [128 feats x 8192 seq]) and V (seq-major via
     PE transpose, augmented with a ones-column for the softmax denominator).
  2. Computes causal attention for its 8 (batch, head) pairs in float32r:
     scoresT = K^T-chunk.T @ Q-strip, exp on ACT, AV+denominator via one
     accumulating matmul against [V | 1].
  3. AllToAll redistributes attention outputs from feature-sharded to
     sequence-sharded; each core then computes its 1024-row block of the
     output projection.
Host wraps: shards weights, classifies mask blocks (skip / keep / masked),
and reassembles the full [4, 2048, 1024] output.
"""

import numpy as np

import concourse.bacc as bacc
import concourse.mybir as mybir
import concourse.tile as tile
from concourse.bass_utils import run_bass_kernel_spmd

F32 = mybir.dt.float32
F32R = mybir.dt.float32r
AF = mybir.ActivationFunctionType
OP = mybir.AluOpType

B, S, D_MODEL, N_HEADS, D_K = 4, 2048, 1024, 16, 64
N_CORES = 8
HPC = N_HEADS // N_CORES          # heads per core = 2
F = HPC * D_K                     # feature slice per core = 128
SEQ = B * S                       # 8192
S1B = 512                         # query-strip width (scores free dim)
S2B = 128                         # key-block height (scores partition dim)
SP = S // S1B                     # 4 strips per batch
C2 = S // S2B                     # 16 key chunks per batch
KC = D_MODEL // 128               # 8 contraction chunks for projections
SC_GLOBAL = SEQ // S1B            # 16 projection seq strips
A_DROP, A_KEEP = -2, -1

_nc_cache = {}


def _build_nc(actions_key, n_masks):
    actions = np.frombuffer(actions_key, dtype=np.int64).reshape(C2, SP)
    nc = bacc.Bacc("TRN2", target_bir_lowering=False, debug=False,
                   num_devices=N_CORES)

    xq = nc.dram_tensor("xq", [D_MODEL, SEQ], F32, kind="ExternalInput")
    xk = nc.dram_tensor("xk", [D_MODEL, SEQ], F32, kind="ExternalInput")
    xv = nc.dram_tensor("xv", [D_MODEL, SEQ], F32, kind="ExternalInput")
    wq = nc.dram_tensor("wq", [D_MODEL, F], F32, kind="ExternalInput")
    wk = nc.dram_tensor("wk", [D_MODEL, F], F32, kind="ExternalInput")
    wv = nc.dram_tensor("wv", [D_MODEL, F], F32, kind="ExternalInput")
    bq = nc.dram_tensor("bq", [F, 1], F32, kind="ExternalInput")
    bk = nc.dram_tensor("bk", [F, 1], F32, kind="ExternalInput")
    bv = nc.dram_tensor("bv", [F, 1], F32, kind="ExternalInput")
    woT = nc.dram_tensor("woT", [D_MODEL, D_MODEL], F32, kind="ExternalInput")
    bo = nc.dram_tensor("bo", [KC, 128, 1], F32, kind="ExternalInput")
    ident = nc.dram_tensor("ident", [128, 128], F32, kind="ExternalInput")
    masks = nc.dram_tensor("masks", [max(n_masks, 1), S2B, S1B], F32,
                           kind="ExternalInput")

    a2a_in = nc.dram_tensor("a2a_in", [N_CORES, F, SEQ // N_CORES], F32)
    a2a_out = nc.dram_tensor("a2a_out", [N_CORES, F, SEQ // N_CORES], F32)
    out_t = nc.dram_tensor("out_t", [D_MODEL, SEQ // N_CORES], F32,
                           kind="ExternalOutput")

    with tile.TileContext(nc) as tc:
        with (
            tc.tile_pool(name="const", bufs=1) as cst,
            tc.tile_pool(name="persist", bufs=1) as per,
            tc.tile_pool(name="xin", bufs=3) as xin,
            tc.tile_pool(name="vtmp", bufs=2) as vtmp,
            tc.tile_pool(name="probs", bufs=4) as prp,
            tc.tile_pool(name="norm", bufs=3) as nrm,
            tc.tile_pool(name="pp_ps", bufs=2, space="PSUM") as pp_ps,
            tc.tile_pool(name="tr_ps", bufs=1, space="PSUM") as tr_ps,
            tc.tile_pool(name="sc_ps", bufs=2, space="PSUM") as sc_ps,
            tc.tile_pool(name="av_ps", bufs=1, space="PSUM") as av_ps,
        ):
            wq_sb = cst.tile([128, KC, F], F32R, tag="wq")
            wk_sb = cst.tile([128, KC, F], F32R, tag="wk")
            wv_sb = cst.tile([128, KC, F], F32R, tag="wv")
            nc.sync.dma_start(wq_sb[:], wq[:].rearrange("(kc p) f -> p kc f", p=128).bitcast(F32R))
            nc.sync.dma_start(wk_sb[:], wk[:].rearrange("(kc p) f -> p kc f", p=128).bitcast(F32R))
            nc.sync.dma_start(wv_sb[:], wv[:].rearrange("(kc p) f -> p kc f", p=128).bitcast(F32R))
            bq_sb = cst.tile([F, 1], F32, tag="bq")
            bk_sb = cst.tile([F, 1], F32, tag="bk")
            bv_sb = cst.tile([F, 1], F32, tag="bv")
            nc.sync.dma_start(bq_sb[:], bq[:])
            nc.sync.dma_start(bk_sb[:], bk[:])
            nc.sync.dma_start(bv_sb[:], bv[:])
            id_sb = cst.tile([128, 128], F32R, tag="id")
            nc.sync.dma_start(id_sb[:], ident[:].bitcast(F32R))
            mk_sb = cst.tile([S2B, max(n_masks, 1), S1B], F32R, tag="mk")
            nc.sync.dma_start(mk_sb[:], masks[:].rearrange("n p f -> p n f").bitcast(F32R))

            qT = per.tile([F, SEQ], F32R, tag="qT")
            kT = per.tile([F, SEQ], F32R, tag="kT")
            # V (seq-major) with per-head ones column: [s2_in_chunk, b, c2, h, dk+1]
            v_aug = per.tile([S2B, B, C2, HPC, D_K + 1], F32R, tag="vaug")
            ones_sb = cst.tile([128, 1], F32, tag="ones")
            nc.vector.memset(ones_sb[:], 1.0)

            def project(x_dram, w_sb, b_sb, sc, dest_slice=None):
                """One [128 feats x 512 seq] projection strip -> psum."""
                ps = pp_ps.tile([128, S1B], F32, tag="pp")
                for kc in range(KC):
                    xt = xin.tile([128, S1B], F32R, tag="xt")
                    nc.sync.dma_start(
                        xt[:],
                        x_dram[kc * 128:(kc + 1) * 128,
                               sc * S1B:(sc + 1) * S1B].bitcast(F32R))
                    nc.tensor.matmul(ps[:], w_sb[:, kc, :], xt[:],
                                     start=(kc == 0), stop=(kc == KC - 1))
                return ps

            for b in range(B):
                # --- projections for batch b ---
                for sc_local in range(SP):
                    sc = b * SP + sc_local
                    ps = project(xq, wq_sb, bq_sb, sc)
                    nc.vector.tensor_scalar_add(
                        qT[:, sc * S1B:(sc + 1) * S1B], ps[:], bq_sb[:, 0:1])
                    ps = project(xk, wk_sb, bk_sb, sc)
                    nc.vector.tensor_scalar_add(
                        kT[:, sc * S1B:(sc + 1) * S1B], ps[:], bk_sb[:, 0:1])
                    ps = project(xv, wv_sb, bv_sb, sc)
                    vt = vtmp.tile([128, S1B], F32R, tag="vt")
                    nc.vector.tensor_scalar_add(vt[:], ps[:], bv_sb[:, 0:1])
                    for j in range(S1B // 128):
                        tp = tr_ps.tile([128, 128], F32R, tag="tr")
                        nc.tensor.transpose(tp[:], vt[:, j * 128:(j + 1) * 128],
                                            id_sb[:])
                        c2 = sc_local * (S1B // 128) + j
                        nc.vector.tensor_copy(
                            v_aug[:, b, c2, :, 0:D_K],
                            tp[:].rearrange("p (h d) -> p h d", h=HPC))
                        nc.vector.tensor_copy(
                            v_aug[:, b, c2, :, D_K:D_K + 1],
                            ones_sb[:, :, None].to_broadcast([S2B, HPC, 1]))

                # --- attention for batch b, both local heads ---
                for lh in range(HPC):
                    r0, r1 = lh * D_K, (lh + 1) * D_K
                    for i1 in range(SP):
                        q_strip = qT[r0:r1, b * S + i1 * S1B: b * S + (i1 + 1) * S1B]
                        kept = [i2 for i2 in range(C2) if actions[i2, i1] != A_DROP]
                        av = av_ps.tile([D_K + 1, S1B], F32, tag="av")
                        n_done = 0
                        while n_done < len(kept):
                            g = kept[n_done:n_done + 2]
                            sc_t = sc_ps.tile([128, S1B * 2], F32, tag="sc")
                            for idx, i2 in enumerate(g):
                                nc.tensor.matmul(
                                    sc_t[:, idx * S1B:(idx + 1) * S1B],
                                    kT[r0:r1, b * S + i2 * S2B: b * S + i2 * S2B + S2B],
                                    q_strip, start=True, stop=True)
                            pr = prp.tile([128, S1B * 2], F32R, tag="pr")
                            nc.scalar.activation(pr[:, 0:len(g) * S1B],
                                                 sc_t[:, 0:len(g) * S1B], AF.Exp)
                            for idx, i2 in enumerate(g):
                                a = actions[i2, i1]
                                prs = pr[:, idx * S1B:(idx + 1) * S1B]
                                if a >= 0:
                                    nc.vector.tensor_tensor(
                                        prs, prs, mk_sb[:, a, :], OP.mult)
                                nc.tensor.matmul(
                                    av[:], v_aug[:, b, i2, lh, :], prs,
                                    start=(n_done + idx == 0),
                                    stop=(n_done + idx == len(kept) - 1))
                            n_done += len(g)
                        rcp = nrm.tile([1, S1B], F32, tag="rcp")
                        nc.vector.reciprocal(rcp[:], av[D_K:D_K + 1, :])
                        bc = nrm.tile([D_K, S1B], F32, tag="bc")
                        nc.gpsimd.partition_broadcast(bc[:], rcp[:], channels=D_K)
                        ob = nrm.tile([D_K, S1B], F32, tag="ob")
                        nc.vector.tensor_tensor(ob[:], av[0:D_K, :], bc[:], OP.mult)
                        s_glob = b * S + i1 * S1B
                        nc.sync.dma_start(
                            a2a_in[s_glob // 1024, r0:r1,
                                   (s_glob % 1024):(s_glob % 1024) + S1B],
                            ob[:])

        nc.gpsimd.collective_compute(
            "AllToAll", OP.bypass,
            ins=[a2a_in[:]], outs=[a2a_out[:]],
            replica_groups=[list(range(N_CORES))])

        with (
            tc.tile_pool(name="oproj", bufs=1) as opr,
            tc.tile_pool(name="ob_sb", bufs=3) as obp,
            tc.tile_pool(name="op_ps", bufs=2, space="PSUM") as op_ps,
        ):
            wo_sb = opr.tile([128, KC, KC, 128], F32R, tag="wo")
            nc.sync.dma_start(
                wo_sb[:],
                woT[:].rearrange("(kc p) (dc f) -> p kc dc f", p=128, f=128).bitcast(F32R))
            bo_sb = opr.tile([128, KC], F32, tag="bo")
            nc.sync.dma_start(bo_sb[:], bo[:].rearrange("d p one -> p (d one)"))
            rhs = opr.tile([128, KC, SEQ // N_CORES], F32R, tag="rhs")
            nc.sync.dma_start(rhs[:], a2a_out[:].rearrange("g p s -> p g s").bitcast(F32R))
            n_sc2 = (SEQ // N_CORES) // S1B
            for dc in range(KC):
                for sc2 in range(n_sc2):
                    ps = op_ps.tile([128, S1B], F32, tag="op")
                    for kc in range(KC):
                        nc.tensor.matmul(
                            ps[:], wo_sb[:, kc, dc, :],
                            rhs[:, kc, sc2 * S1B:(sc2 + 1) * S1B],
                            start=(kc == 0), stop=(kc == KC - 1))
                    ob = obp.tile([128, S1B], F32, tag="obt")
                    nc.vector.tensor_scalar_add(ob[:], ps[:], bo_sb[:, dc:dc + 1])
                    nc.sync.dma_start(
                        out_t[dc * 128:(dc + 1) * 128,
                              sc2 * S1B:(sc2 + 1) * S1B], ob[:])

    nc.finalize()
    return nc


def _classify_mask(mask):
    """Block-classify mask[0,0] on the scoresT grid: per (key-chunk i2,
    query-strip i1) -> drop / keep / index of a unique [128, 512] 0/1 tile."""
    m2 = np.asarray(mask)[0, 0] != 0  # [S, S], m2[q, k]
    actions = np.full((C2, SP), A_DROP, dtype=np.int64)
    uniq, tiles = {}, []
    for i2 in range(C2):
        for i1 in range(SP):
            blk = m2[i1 * S1B:(i1 + 1) * S1B, i2 * S2B:(i2 + 1) * S2B].T
            if blk.all():
                actions[i2, i1] = A_KEEP
            elif blk.any():
                key = blk.tobytes()
                if key not in uniq:
                    uniq[key] = len(tiles)
                    tiles.append(np.ascontiguousarray(blk, dtype=np.float32))
                actions[i2, i1] = uniq[key]
    arr = (np.stack(tiles) if tiles
           else np.zeros((1, S2B, S1B), dtype=np.float32))
    return actions, arr


def _prep(inputs):
    q = np.asarray(inputs["query"], dtype=np.float32).reshape(SEQ, D_MODEL)
    k = np.asarray(inputs["key"], dtype=np.float32).reshape(SEQ, D_MODEL)
    v = np.asarray(inputs["value"], dtype=np.float32).reshape(SEQ, D_MODEL)
    xq = np.ascontiguousarray(q.T)
    xk = np.ascontiguousarray(k.T)
    xv = np.ascontiguousarray(v.T)

    Wq = np.asarray(inputs["Wq"], dtype=np.float32)
    Wk = np.asarray(inputs["Wk"], dtype=np.float32)
    Wv = np.asarray(inputs["Wv"], dtype=np.float32)
    Wo = np.asarray(inputs["Wo"], dtype=np.float32)
    bq = np.asarray(inputs["bq"], dtype=np.float32)
    bk = np.asarray(inputs["bk"], dtype=np.float32)
    bv = np.asarray(inputs["bv"], dtype=np.float32)
    bo = np.asarray(inputs["bo"], dtype=np.float32)

    scale = 1.0 / np.sqrt(D_K)
    actions, mask_tiles = _classify_mask(inputs["mask"])

    # exp-overflow guard for the no-max-subtract softmax (Cauchy-Schwarz bound)
    qn = q @ Wq.T + bq
    kn = k @ Wk.T + bk
    qmax = np.linalg.norm(qn.reshape(SEQ, N_HEADS, D_K), axis=-1).max()
    kmax = np.linalg.norm(kn.reshape(SEQ, N_HEADS, D_K), axis=-1).max()
    assert scale * qmax * kmax < 80.0, "score bound too large for exp without max-subtraction"

    shared = {
        "xq": xq, "xk": xk, "xv": xv,
        "woT": np.ascontiguousarray(Wo.T),
        "bo": np.ascontiguousarray(bo.reshape(KC, 128, 1)),
        "ident": np.eye(128, dtype=np.float32),
        "masks": mask_tiles,
    }
    in_maps = []
    for c in range(N_CORES):
        sl = slice(c * F, (c + 1) * F)
        m = dict(shared)
        m["wq"] = np.ascontiguousarray((Wq[sl] * scale).T)
        m["wk"] = np.ascontiguousarray(Wk[sl].T)
        m["wv"] = np.ascontiguousarray(Wv[sl].T)
        m["bq"] = np.ascontiguousarray((bq[sl] * scale).reshape(F, 1))
        m["bk"] = np.ascontiguousarray(bk[sl].reshape(F, 1))
        m["bv"] = np.ascontiguousarray(bv[sl].reshape(F, 1))
        in_maps.append(m)
    return in_maps, actions, mask_tiles


def _run(inputs, trace=False, trace_cores=None):
    in_maps, actions, mask_tiles = _prep(inputs)
    key = (actions.tobytes(), len(mask_tiles))
    if key not in _nc_cache:
        _nc_cache[key] = _build_nc(key[0], key[1])
    nc = _nc_cache[key]
    res = run_bass_kernel_spmd(nc, in_maps, list(range(N_CORES)),
                               trace=trace, trace_cores=trace_cores)
    blk = SEQ // N_CORES
    out = np.empty((SEQ, D_MODEL), dtype=np.float32)
    for c in range(N_CORES):
        out[c * blk:(c + 1) * blk] = res.results[c]["out_t"].T
    return out.reshape(B, S, D_MODEL), res


def kernel(**inputs) -> np.ndarray:
    out, _ = _run(inputs)
    return out


# revision 9
# speedup vs baseline: 1.4824x; 1.4824x over previous
"""Trainium2 Bass kernel for nn_MultiHeadAttention (B=4, S=2048, D=1024, H=16).

Sharding: tensor-parallel over heads (2 heads per core, 8 cores). Each core:
  1. Projects Q/K (feature-major, [128 feats x 8192 seq]) and V (seq-major via
     PE transpose, augmented with a ones-column for the softmax denominator).
  2. Computes causal attention for its 8 (batch, head) pairs in bf16 with
     fp32 PSUM accumulation: scoresT = K-chunk @ Q-strip, exp on ACT,
     AV+denominator via one accumulating matmul against [V | 1].
  3. AllToAll redistributes attention outputs from feature-sharded to
     sequence-sharded; each core then computes its 1024-row block of the
     output projection (float32r for accuracy).
Host wraps: shards weights, classifies mask blocks (skip / keep / masked),
and reassembles the full [4, 2048, 1024] output.
"""

import ml_dtypes
import numpy as np

import concourse.bacc as bacc
import concourse.mybir as mybir
import concourse.tile as tile
from concourse.bass_utils import run_bass_kernel_spmd

F32 = mybir.dt.float32
F32R = mybir.dt.float32r
BF16 = mybir.dt.bfloat16
AF = mybir.ActivationFunctionType
OP = mybir.AluOpType

B, S, D_MODEL, N_HEADS, D_K = 4, 2048, 1024, 16, 64
N_CORES = 8
HPC = N_HEADS // N_CORES          # heads per core = 2
F = HPC * D_K                     # feature slice per core = 128
SEQ = B * S                       # 8192
S1B = 512                         # query-strip width (scores free dim)
S2B = 128                         # key-block height (scores partition dim)
SP = S // S1B                     # 4 strips per batch
C2 = S // S2B                     # 16 key chunks per batch
KC = D_MODEL // 128               # 8 contraction chunks for projections
SC_GLOBAL = SEQ // S1B            # 16 projection seq strips
A_DROP, A_KEEP = -2, -1

_nc_cache = {}


def _build_nc(actions_key, n_masks):
    actions = np.frombuffer(actions_key, dtype=np.int64).reshape(C2, SP)
    nc = bacc.Bacc("TRN2", target_bir_lowering=False, debug=False,
                   num_devices=N_CORES)

    xq = nc.dram_tensor("xq", [D_MODEL, SEQ], BF16, kind="ExternalInput")
    xk = nc.dram_tensor("xk", [D_MODEL, SEQ], BF16, kind="ExternalInput")
    xv = nc.dram_tensor("xv", [D_MODEL, SEQ], BF16, kind="ExternalInput")
    wq = nc.dram_tensor("wq", [D_MODEL, F], BF16, kind="ExternalInput")
    wk = nc.dram_tensor("wk", [D_MODEL, F], BF16, kind="ExternalInput")
    wv = nc.dram_tensor("wv", [D_MODEL, F], BF16, kind="ExternalInput")
    bq = nc.dram_tensor("bq", [F, 1], F32, kind="ExternalInput")
    bk = nc.dram_tensor("bk", [F, 1], F32, kind="ExternalInput")
    bv = nc.dram_tensor("bv", [F, 1], F32, kind="ExternalInput")
    woT = nc.dram_tensor("woT", [D_MODEL, D_MODEL], F32, kind="ExternalInput")
    bo = nc.dram_tensor("bo", [KC, 128, 1], F32, kind="ExternalInput")
    ident = nc.dram_tensor("ident", [128, 128], BF16, kind="ExternalInput")
    masks = nc.dram_tensor("masks", [max(n_masks, 1), S2B, S1B], BF16,
                           kind="ExternalInput")

    a2a_in = nc.dram_tensor("a2a_in", [N_CORES, F, SEQ // N_CORES], F32)
    a2a_out = nc.dram_tensor("a2a_out", [N_CORES, F, SEQ // N_CORES], F32)
    out_t = nc.dram_tensor("out_t", [D_MODEL, SEQ // N_CORES], F32,
                           kind="ExternalOutput")

    with tile.TileContext(nc) as tc:
        with (
            tc.tile_pool(name="const", bufs=1) as cst,
            tc.tile_pool(name="persist", bufs=1) as per,
            tc.tile_pool(name="xin", bufs=10) as xin,
            tc.tile_pool(name="vtmp", bufs=2) as vtmp,
            tc.tile_pool(name="probs", bufs=6) as prp,
            tc.tile_pool(name="norm", bufs=3) as nrm,
            tc.tile_pool(name="pp_ps", bufs=2, space="PSUM") as pp_ps,
            tc.tile_pool(name="tr_ps", bufs=1, space="PSUM") as tr_ps,
            tc.tile_pool(name="sc_ps", bufs=2, space="PSUM") as sc_ps,
            tc.tile_pool(name="av_ps", bufs=1, space="PSUM") as av_ps,
        ):
            wq_sb = cst.tile([128, KC, F], BF16, tag="wq")
            wk_sb = cst.tile([128, KC, F], BF16, tag="wk")
            wv_sb = cst.tile([128, KC, F], BF16, tag="wv")
            nc.sync.dma_start(wq_sb[:], wq[:].rearrange("(kc p) f -> p kc f", p=128))
            nc.sync.dma_start(wk_sb[:], wk[:].rearrange("(kc p) f -> p kc f", p=128))
            nc.sync.dma_start(wv_sb[:], wv[:].rearrange("(kc p) f -> p kc f", p=128))
            bq_sb = cst.tile([F, 1], F32, tag="bq")
            bk_sb = cst.tile([F, 1], F32, tag="bk")
            bv_sb = cst.tile([F, 1], F32, tag="bv")
            nc.sync.dma_start(bq_sb[:], bq[:])
            nc.sync.dma_start(bk_sb[:], bk[:])
            nc.sync.dma_start(bv_sb[:], bv[:])
            id_sb = cst.tile([128, 128], BF16, tag="id")
            nc.sync.dma_start(id_sb[:], ident[:])
            mk_sb = cst.tile([S2B, max(n_masks, 1), S1B], BF16, tag="mk")
            nc.sync.dma_start(mk_sb[:], masks[:].rearrange("n p f -> p n f"))

            qT = per.tile([F, SEQ], BF16, tag="qT")
            kT = per.tile([F, SEQ], BF16, tag="kT")
            # V (seq-major) with per-head ones column: [s2_in_chunk, b, c2, h, dk+1]
            v_aug = per.tile([S2B, B, C2, HPC, D_K + 1], BF16, tag="vaug")
            ones_sb = cst.tile([128, 1], F32, tag="ones")
            nc.vector.memset(ones_sb[:], 1.0)

            for b in range(B):
                # --- projections for batch b (per tensor: load 8 k-chunks
                # of the batch as [128, 2048] tiles, project 4 strips) ---
                for name, x_dram, w_sb, b_sb in (
                    ("q", xq, wq_sb, bq_sb),
                    ("k", xk, wk_sb, bk_sb),
                    ("v", xv, wv_sb, bv_sb),
                ):
                    xts = []
                    for kc in range(KC):
                        xt = xin.tile([128, S], BF16, tag="xt")
                        nc.sync.dma_start(
                            xt[:], x_dram[kc * 128:(kc + 1) * 128,
                                          b * S:(b + 1) * S])
                        xts.append(xt)
                    for sc_local in range(SP):
                        sl = slice(sc_local * S1B, (sc_local + 1) * S1B)
                        gsl = slice(b * S + sc_local * S1B,
                                    b * S + (sc_local + 1) * S1B)
                        ps = pp_ps.tile([128, S1B], F32, tag="pp")
                        for kc in range(KC):
                            nc.tensor.matmul(ps[:], w_sb[:, kc, :], xts[kc][:, sl],
                                             start=(kc == 0), stop=(kc == KC - 1))
                        if name == "q":
                            nc.vector.tensor_scalar_add(qT[:, gsl], ps[:], b_sb[:, 0:1])
                        elif name == "k":
                            nc.vector.tensor_scalar_add(kT[:, gsl], ps[:], b_sb[:, 0:1])
                        else:
                            vt = vtmp.tile([128, S1B], BF16, tag="vt")
                            nc.vector.tensor_scalar_add(vt[:], ps[:], b_sb[:, 0:1])
                            for j in range(S1B // 128):
                                tp = tr_ps.tile([128, 128], BF16, tag="tr")
                                nc.tensor.transpose(tp[:], vt[:, j * 128:(j + 1) * 128],
                                                    id_sb[:])
                                c2 = sc_local * (S1B // 128) + j
                                nc.vector.tensor_copy(
                                    v_aug[:, b, c2, :, 0:D_K],
                                    tp[:].rearrange("p (h d) -> p h d", h=HPC))
                                nc.vector.tensor_copy(
                                    v_aug[:, b, c2, :, D_K:D_K + 1],
                                    ones_sb[:, :, None].to_broadcast([S2B, HPC, 1]))

                # --- attention for batch b, both local heads ---
                for lh in range(HPC):
                    r0, r1 = lh * D_K, (lh + 1) * D_K
                    for i1 in range(SP):
                        q_strip = qT[r0:r1, b * S + i1 * S1B: b * S + (i1 + 1) * S1B]
                        kept = [i2 for i2 in range(C2) if actions[i2, i1] != A_DROP]
                        av = av_ps.tile([D_K + 1, S1B], F32, tag="av")
                        n_done = 0
                        while n_done < len(kept):
                            g = kept[n_done:n_done + 2]
                            sc_t = sc_ps.tile([128, S1B * 2], F32, tag="sc")
                            for idx, i2 in enumerate(g):
                                nc.tensor.matmul(
                                    sc_t[:, idx * S1B:(idx + 1) * S1B],
                                    kT[r0:r1, b * S + i2 * S2B: b * S + i2 * S2B + S2B],
                                    q_strip, start=True, stop=True)
                            pr = prp.tile([128, S1B * 2], BF16, tag="pr")
                            nc.scalar.activation(pr[:, 0:len(g) * S1B],
                                                 sc_t[:, 0:len(g) * S1B], AF.Exp)
                            for idx, i2 in enumerate(g):
                                a = actions[i2, i1]
                                prs = pr[:, idx * S1B:(idx + 1) * S1B]
                                if a >= 0:
                                    nc.vector.tensor_tensor(
                                        prs, prs, mk_sb[:, a, :], OP.mult)
                                nc.tensor.matmul(
                                    av[:], v_aug[:, b, i2, lh, :], prs,
                                    start=(n_done + idx == 0),
                                    stop=(n_done + idx == len(kept) - 1))
                            n_done += len(g)
                        rcp = nrm.tile([1, S1B], F32, tag="rcp")
                        nc.vector.reciprocal(rcp[:], av[D_K:D_K + 1, :])
                        bc = nrm.tile([D_K, S1B], F32, tag="bc")
                        nc.gpsimd.partition_broadcast(bc[:], rcp[:], channels=D_K)
                        ob = nrm.tile([D_K, S1B], F32, tag="ob")
                        nc.vector.tensor_tensor(ob[:], av[0:D_K, :], bc[:], OP.mult)
                        s_glob = b * S + i1 * S1B
                        nc.sync.dma_start(
                            a2a_in[s_glob // 1024, r0:r1,
                                   (s_glob % 1024):(s_glob % 1024) + S1B],
                            ob[:])

        nc.gpsimd.collective_compute(
            "AllToAll", OP.bypass,
            ins=[a2a_in[:]], outs=[a2a_out[:]],
            replica_groups=[list(range(N_CORES))])

        with (
            tc.tile_pool(name="oproj", bufs=1) as opr,
            tc.tile_pool(name="ob_sb", bufs=3) as obp,
            tc.tile_pool(name="op_ps", bufs=2, space="PSUM") as op_ps,
        ):
            wo_sb = opr.tile([128, KC, KC, 128], F32R, tag="wo")
            nc.sync.dma_start(
                wo_sb[:],
                woT[:].rearrange("(kc p) (dc f) -> p kc dc f", p=128, f=128).bitcast(F32R))
            bo_sb = opr.tile([128, KC], F32, tag="bo")
            nc.sync.dma_start(bo_sb[:], bo[:].rearrange("d p one -> p (d one)"))
            rhs = opr.tile([128, KC, SEQ // N_CORES], F32R, tag="rhs")
            nc.sync.dma_start(rhs[:], a2a_out[:].rearrange("g p s -> p g s").bitcast(F32R))
            n_sc2 = (SEQ // N_CORES) // S1B
            for dc in range(KC):
                for sc2 in range(n_sc2):
                    ps = op_ps.tile([128, S1B], F32, tag="op")
                    for kc in range(KC):
                        nc.tensor.matmul(
                            ps[:], wo_sb[:, kc, dc, :],
                            rhs[:, kc, sc2 * S1B:(sc2 + 1) * S1B],
                            start=(kc == 0), stop=(kc == KC - 1))
                    ob = obp.tile([128, S1B], F32, tag="obt")
                    nc.vector.tensor_scalar_add(ob[:], ps[:], bo_sb[:, dc:dc + 1])
                    nc.sync.dma_start(
                        out_t[dc * 128:(dc + 1) * 128,
                              sc2 * S1B:(sc2 + 1) * S1B], ob[:])

    nc.finalize()
    return nc


def _classify_mask(mask):
    """Block-classify mask[0,0] on the scoresT grid: per (key-chunk i2,
    query-strip i1) -> drop / keep / index of a unique [128, 512] 0/1 tile."""
    m2 = np.asarray(mask)[0, 0] != 0  # [S, S], m2[q, k]
    actions = np.full((C2, SP), A_DROP, dtype=np.int64)
    uniq, tiles = {}, []
    for i2 in range(C2):
        for i1 in range(SP):
            blk = m2[i1 * S1B:(i1 + 1) * S1B, i2 * S2B:(i2 + 1) * S2B].T
            if blk.all():
                actions[i2, i1] = A_KEEP
            elif blk.any():
                key = blk.tobytes()
                if key not in uniq:
                    uniq[key] = len(tiles)
                    tiles.append(np.ascontiguousarray(blk).astype(ml_dtypes.bfloat16))
                actions[i2, i1] = uniq[key]
    arr = (np.stack(tiles) if tiles
           else np.zeros((1, S2B, S1B), dtype=ml_dtypes.bfloat16))
    return actions, arr


def _prep(inputs):
    q = np.asarray(inputs["query"], dtype=np.float32).reshape(SEQ, D_MODEL)
    k = np.asarray(inputs["key"], dtype=np.float32).reshape(SEQ, D_MODEL)
    v = np.asarray(inputs["value"], dtype=np.float32).reshape(SEQ, D_MODEL)
    bf = ml_dtypes.bfloat16
    xq = np.ascontiguousarray(q.T).astype(bf)
    xk = np.ascontiguousarray(k.T).astype(bf)
    xv = np.ascontiguousarray(v.T).astype(bf)

    Wq = np.asarray(inputs["Wq"], dtype=np.float32)
    Wk = np.asarray(inputs["Wk"], dtype=np.float32)
    Wv = np.asarray(inputs["Wv"], dtype=np.float32)
    Wo = np.asarray(inputs["Wo"], dtype=np.float32)
    bq = np.asarray(inputs["bq"], dtype=np.float32)
    bk = np.asarray(inputs["bk"], dtype=np.float32)
    bv = np.asarray(inputs["bv"], dtype=np.float32)
    bo = np.asarray(inputs["bo"], dtype=np.float32)

    scale = 1.0 / np.sqrt(D_K)
    actions, mask_tiles = _classify_mask(inputs["mask"])

    # exp-overflow guard for the no-max-subtract softmax (Cauchy-Schwarz bound)
    qn = q @ Wq.T + bq
    kn = k @ Wk.T + bk
    qmax = np.linalg.norm(qn.reshape(SEQ, N_HEADS, D_K), axis=-1).max()
    kmax = np.linalg.norm(kn.reshape(SEQ, N_HEADS, D_K), axis=-1).max()
    assert scale * qmax * kmax < 80.0, "score bound too large for exp without max-subtraction"

    shared = {
        "xq": xq, "xk": xk, "xv": xv,
        "woT": np.ascontiguousarray(Wo.T),
        "bo": np.ascontiguousarray(bo.reshape(KC, 128, 1)),
        "ident": np.eye(128, dtype=np.float32).astype(bf),
        "masks": mask_tiles,
    }
    in_maps = []
    for c in range(N_CORES):
        sl = slice(c * F, (c + 1) * F)
        m = dict(shared)
        m["wq"] = np.ascontiguousarray((Wq[sl] * scale).T).astype(bf)
        m["wk"] = np.ascontiguousarray(Wk[sl].T).astype(bf)
        m["wv"] = np.ascontiguousarray(Wv[sl].T).astype(bf)
        m["bq"] = np.ascontiguousarray((bq[sl] * scale).reshape(F, 1))
        m["bk"] = np.ascontiguousarray(bk[sl].reshape(F, 1))
        m["bv"] = np.ascontiguousarray(bv[sl].reshape(F, 1))
        in_maps.append(m)
    return in_maps, actions, mask_tiles


def _run(inputs, trace=False, trace_cores=None):
    in_maps, actions, mask_tiles = _prep(inputs)
    key = (actions.tobytes(), len(mask_tiles))
    if key not in _nc_cache:
        _nc_cache[key] = _build_nc(key[0], key[1])
    nc = _nc_cache[key]
    res = run_bass_kernel_spmd(nc, in_maps, list(range(N_CORES)),
                               trace=trace, trace_cores=trace_cores)
    blk = SEQ // N_CORES
    out = np.empty((SEQ, D_MODEL), dtype=np.float32)
    for c in range(N_CORES):
        out[c * blk:(c + 1) * blk] = res.results[c]["out_t"].T
    return out.reshape(B, S, D_MODEL), res


def kernel(**inputs) -> np.ndarray:
    out, _ = _run(inputs)
    return out


# revision 14
# speedup vs baseline: 1.5189x; 1.0246x over previous
"""Trainium2 Bass kernel for nn_MultiHeadAttention (B=4, S=2048, D=1024, H=16).

Sharding: tensor-parallel over heads (2 heads per core, 8 cores). Each core:
  1. Projects Q/K (feature-major, [128 feats x 8192 seq]) and V (seq-major via
     PE transpose, augmented with a ones-column for the softmax denominator).
  2. Computes causal attention for its 8 (batch, head) pairs in bf16 with
     fp32 PSUM accumulation: scoresT = K-chunk @ Q-strip, exp on ACT,
     AV+denominator via one accumulating matmul against [V | 1].
  3. AllToAll redistributes attention outputs from feature-sharded to
     sequence-sharded; each core then computes its 1024-row block of the
     output projection (float32r for accuracy).
Host wraps: shards weights, classifies mask blocks (skip / keep / masked),
and reassembles the full [4, 2048, 1024] output.
"""

import ml_dtypes
import numpy as np

import concourse.bacc as bacc
import concourse.mybir as mybir
import concourse.tile as tile
from concourse.bass_utils import run_bass_kernel_spmd

F32 = mybir.dt.float32
F32R = mybir.dt.float32r
BF16 = mybir.dt.bfloat16
AF = mybir.ActivationFunctionType
OP = mybir.AluOpType

B, S, D_MODEL, N_HEADS, D_K = 4, 2048, 1024, 16, 64
N_CORES = 8
HPC = N_HEADS // N_CORES          # heads per core = 2
F = HPC * D_K                     # feature slice per core = 128
SEQ = B * S                       # 8192
S1B = 512                         # query-strip width (scores free dim)
S2B = 128                         # key-block height (scores partition dim)
SP = S // S1B                     # 4 strips per batch
C2 = S // S2B                     # 16 key chunks per batch
KC = D_MODEL // 128               # 8 contraction chunks for projections
SC_GLOBAL = SEQ // S1B            # 16 projection seq strips
A_DROP, A_KEEP = -2, -1

_nc_cache = {}


def _build_nc(actions_key, n_masks):
    actions = np.frombuffer(actions_key, dtype=np.int64).reshape(C2, SP)
    nc = bacc.Bacc("TRN2", target_bir_lowering=False, debug=False,
                   num_devices=N_CORES)

    xq = nc.dram_tensor("xq", [D_MODEL, SEQ], BF16, kind="ExternalInput")
    xk = nc.dram_tensor("xk", [D_MODEL, SEQ], BF16, kind="ExternalInput")
    xv = nc.dram_tensor("xv", [D_MODEL, SEQ], BF16, kind="ExternalInput")
    wq = nc.dram_tensor("wq", [D_MODEL, F], BF16, kind="ExternalInput")
    wk = nc.dram_tensor("wk", [D_MODEL, F], BF16, kind="ExternalInput")
    wv = nc.dram_tensor("wv", [D_MODEL, F], BF16, kind="ExternalInput")
    bq = nc.dram_tensor("bq", [F, 1], F32, kind="ExternalInput")
    bk = nc.dram_tensor("bk", [F, 1], F32, kind="ExternalInput")
    bv = nc.dram_tensor("bv", [F, 1], F32, kind="ExternalInput")
    woT = nc.dram_tensor("woT", [D_MODEL, D_MODEL], F32, kind="ExternalInput")
    bo = nc.dram_tensor("bo", [KC, 128, 1], F32, kind="ExternalInput")
    ident = nc.dram_tensor("ident", [128, 128], BF16, kind="ExternalInput")
    masks = nc.dram_tensor("masks", [max(n_masks, 1), S2B, S1B], BF16,
                           kind="ExternalInput")

    a2a_in = nc.dram_tensor("a2a_in", [N_CORES, F, SEQ // N_CORES], F32)
    a2a_out = nc.dram_tensor("a2a_out", [N_CORES, F, SEQ // N_CORES], F32)
    out_t = nc.dram_tensor("out_t", [D_MODEL, SEQ // N_CORES], F32,
                           kind="ExternalOutput")

    with tile.TileContext(nc) as tc:
      with tc.tile_pool(name="oproj_w", bufs=1) as opw:
        wo_sb = opw.tile([128, KC, KC, 128], F32R, tag="wo")
        nc.sync.dma_start(
            wo_sb[:],
            woT[:].rearrange("(kc p) (dc f) -> p kc dc f", p=128, f=128).bitcast(F32R))
        bo_sb = opw.tile([128, KC], F32, tag="bo")
        nc.sync.dma_start(bo_sb[:], bo[:].rearrange("d p one -> p (d one)"))
        with (
            tc.tile_pool(name="const", bufs=1) as cst,
            tc.tile_pool(name="persist", bufs=1) as per,
            tc.tile_pool(name="xin", bufs=10) as xin,
            tc.tile_pool(name="vtmp", bufs=2) as vtmp,
            tc.tile_pool(name="probs", bufs=6) as prp,
            tc.tile_pool(name="norm", bufs=3) as nrm,
            tc.tile_pool(name="pp_ps", bufs=1, space="PSUM") as pp_ps,
            tc.tile_pool(name="tr_ps", bufs=1, space="PSUM") as tr_ps,
            tc.tile_pool(name="sc_ps", bufs=2, space="PSUM") as sc_ps,
            tc.tile_pool(name="av_ps", bufs=2, space="PSUM") as av_ps,
        ):
            wq_sb = cst.tile([128, KC, F], BF16, tag="wq")
            wk_sb = cst.tile([128, KC, F], BF16, tag="wk")
            wv_sb = cst.tile([128, KC, F], BF16, tag="wv")
            nc.sync.dma_start(wq_sb[:], wq[:].rearrange("(kc p) f -> p kc f", p=128))
            nc.sync.dma_start(wk_sb[:], wk[:].rearrange("(kc p) f -> p kc f", p=128))
            nc.sync.dma_start(wv_sb[:], wv[:].rearrange("(kc p) f -> p kc f", p=128))
            bq_sb = cst.tile([F, 1], F32, tag="bq")
            bk_sb = cst.tile([F, 1], F32, tag="bk")
            bv_sb = cst.tile([F, 1], F32, tag="bv")
            nc.sync.dma_start(bq_sb[:], bq[:])
            nc.sync.dma_start(bk_sb[:], bk[:])
            nc.sync.dma_start(bv_sb[:], bv[:])
            id_sb = cst.tile([128, 128], BF16, tag="id")
            nc.sync.dma_start(id_sb[:], ident[:])
            mk_sb = cst.tile([S2B, max(n_masks, 1), S1B], BF16, tag="mk")
            nc.sync.dma_start(mk_sb[:], masks[:].rearrange("n p f -> p n f"))

            qT = per.tile([F, SEQ], BF16, tag="qT")
            kT = per.tile([F, SEQ], BF16, tag="kT")
            # V (seq-major) with per-head ones column: [s2_in_chunk, b, c2, h, dk+1]
            v_aug = per.tile([S2B, B, C2, HPC, D_K + 1], BF16, tag="vaug")
            ones_sb = cst.tile([128, 1], F32, tag="ones")
            nc.vector.memset(ones_sb[:], 1.0)

            for b in range(B):
                # --- projections for batch b (per tensor: load 8 k-chunks
                # of the batch as [128, 2048] tiles, project 4 strips) ---
                for name, x_dram, w_sb, b_sb in (
                    ("q", xq, wq_sb, bq_sb),
                    ("k", xk, wk_sb, bk_sb),
                    ("v", xv, wv_sb, bv_sb),
                ):
                    xts = []
                    for kc in range(KC):
                        xt = xin.tile([128, S], BF16, tag="xt")
                        nc.sync.dma_start(
                            xt[:], x_dram[kc * 128:(kc + 1) * 128,
                                          b * S:(b + 1) * S])
                        xts.append(xt)
                    for sc_local in range(SP):
                        sl = slice(sc_local * S1B, (sc_local + 1) * S1B)
                        gsl = slice(b * S + sc_local * S1B,
                                    b * S + (sc_local + 1) * S1B)
                        ps = pp_ps.tile([128, S1B], F32, tag="pp")
                        for kc in range(KC):
                            nc.tensor.matmul(ps[:], w_sb[:, kc, :], xts[kc][:, sl],
                                             start=(kc == 0), stop=(kc == KC - 1))
                        if name == "q":
                            nc.vector.tensor_scalar_add(qT[:, gsl], ps[:], b_sb[:, 0:1])
                        elif name == "k":
                            nc.vector.tensor_scalar_add(kT[:, gsl], ps[:], b_sb[:, 0:1])
                        else:
                            vt = vtmp.tile([128, S1B], BF16, tag="vt")
                            nc.vector.tensor_scalar_add(vt[:], ps[:], b_sb[:, 0:1])
                            for j in range(S1B // 128):
                                tp = tr_ps.tile([128, 128], BF16, tag="tr")
                                nc.tensor.transpose(tp[:], vt[:, j * 128:(j + 1) * 128],
                                                    id_sb[:])
                                c2 = sc_local * (S1B // 128) + j
                                nc.vector.tensor_copy(
                                    v_aug[:, b, c2, :, 0:D_K],
                                    tp[:].rearrange("p (h d) -> p h d", h=HPC))
                                nc.vector.tensor_copy(
                                    v_aug[:, b, c2, :, D_K:D_K + 1],
                                    ones_sb[:, :, None].to_broadcast([S2B, HPC, 1]))

                # --- attention for batch b, both local heads interleaved ---
                for i1 in range(SP):
                    kept = [i2 for i2 in range(C2) if actions[i2, i1] != A_DROP]
                    avs = []
                    for _lh in range(HPC):
                        av_t = av_ps.tile([D_K + 1, S1B], F32, tag="av")
                        avs.append(av_t)
                    n_done = 0
                    while n_done < len(kept):
                        g = kept[n_done:n_done + 2]
                        prs_all = []
                        for lh in range(HPC):
                            r0, r1 = lh * D_K, (lh + 1) * D_K
                            q_strip = qT[r0:r1,
                                         b * S + i1 * S1B: b * S + (i1 + 1) * S1B]
                            sc_t = sc_ps.tile([128, S1B * 2], F32, tag="sc")
                            for idx, i2 in enumerate(g):
                                nc.tensor.matmul(
                                    sc_t[:, idx * S1B:(idx + 1) * S1B],
                                    kT[r0:r1, b * S + i2 * S2B: b * S + i2 * S2B + S2B],
                                    q_strip, start=True, stop=True)
                            pr = prp.tile([128, S1B * 2], BF16, tag="pr")
                            nc.scalar.activation(pr[:, 0:len(g) * S1B],
                                                 sc_t[:, 0:len(g) * S1B], AF.Exp)
                            prs_all.append(pr)
                        for lh in range(HPC):
                            for idx, i2 in enumerate(g):
                                a = actions[i2, i1]
                                prs = prs_all[lh][:, idx * S1B:(idx + 1) * S1B]
                                if a >= 0:
                                    nc.vector.tensor_tensor(
                                        prs, prs, mk_sb[:, a, :], OP.mult)
                                nc.tensor.matmul(
                                    avs[lh][:], v_aug[:, b, i2, lh, :], prs,
                                    start=(n_done + idx == 0),
                                    stop=(n_done + idx == len(kept) - 1))
                        n_done += len(g)
                    for lh in range(HPC):
                        r0, r1 = lh * D_K, (lh + 1) * D_K
                        av = avs[lh]
                        tln = nrm.tile([1, S1B], F32, tag="tln")
                        nc.scalar.activation(tln[:], av[D_K:D_K + 1, :], AF.Ln)
                        rcp = nrm.tile([1, S1B], F32, tag="rcp")
                        nc.scalar.activation(rcp[:], tln[:], AF.Exp, scale=-1.0)
                        bc = nrm.tile([D_K, S1B], F32, tag="bc")
                        nc.gpsimd.partition_broadcast(bc[:], rcp[:], channels=D_K)
                        ob = nrm.tile([D_K, S1B], F32, tag="ob")
                        nc.vector.tensor_tensor(ob[:], av[0:D_K, :], bc[:], OP.mult)
                        s_glob = b * S + i1 * S1B
                        nc.sync.dma_start(
                            a2a_in[s_glob // 1024, r0:r1,
                                   (s_glob % 1024):(s_glob % 1024) + S1B],
                            ob[:])

        nc.gpsimd.collective_compute(
            "AllToAll", OP.bypass,
            ins=[a2a_in[:]], outs=[a2a_out[:]],
            replica_groups=[list(range(N_CORES))])

        with (
            tc.tile_pool(name="oproj", bufs=1) as opr,
            tc.tile_pool(name="ob_sb", bufs=3) as obp,
            tc.tile_pool(name="op_ps", bufs=2, space="PSUM") as op_ps,
        ):
            rhs = opr.tile([128, KC, SEQ // N_CORES], F32R, tag="rhs")
            nc.sync.dma_start(rhs[:], a2a_out[:].rearrange("g p s -> p g s").bitcast(F32R))
            n_sc2 = (SEQ // N_CORES) // S1B
            for dc in range(KC):
                for sc2 in range(n_sc2):
                    ps = op_ps.tile([128, S1B], F32, tag="op")
                    for kc in range(KC):
                        nc.tensor.matmul(
                            ps[:], wo_sb[:, kc, dc, :],
                            rhs[:, kc, sc2 * S1B:(sc2 + 1) * S1B],
                            start=(kc == 0), stop=(kc == KC - 1))
                    ob = obp.tile([128, S1B], F32, tag="obt")
                    nc.vector.tensor_scalar_add(ob[:], ps[:], bo_sb[:, dc:dc + 1])
                    nc.sync.dma_start(
                        out_t[dc * 128:(dc + 1) * 128,
                              sc2 * S1B:(sc2 + 1) * S1B], ob[:])

    nc.finalize()
    return nc


def _classify_mask(mask):
    """Block-classify mask[0,0] on the scoresT grid: per (key-chunk i2,
    query-strip i1) -> drop / keep / index of a unique [128, 512] 0/1 tile."""
    m2 = np.asarray(mask)[0, 0] != 0  # [S, S], m2[q, k]
    actions = np.full((C2, SP), A_DROP, dtype=np.int64)
    uniq, tiles = {}, []
    for i2 in range(C2):
        for i1 in range(SP):
            blk = m2[i1 * S1B:(i1 + 1) * S1B, i2 * S2B:(i2 + 1) * S2B].T
            if blk.all():
                actions[i2, i1] = A_KEEP
            elif blk.any():
                key = blk.tobytes()
                if key not in uniq:
                    uniq[key] = len(tiles)
                    tiles.append(np.ascontiguousarray(blk).astype(ml_dtypes.bfloat16))
                actions[i2, i1] = uniq[key]
    arr = (np.stack(tiles) if tiles
           else np.zeros((1, S2B, S1B), dtype=ml_dtypes.bfloat16))
    return actions, arr


def _prep(inputs):
    q = np.asarray(inputs["query"], dtype=np.float32).reshape(SEQ, D_MODEL)
    k = np.asarray(inputs["key"], dtype=np.float32).reshape(SEQ, D_MODEL)
    v = np.asarray(inputs["value"], dtype=np.float32).reshape(SEQ, D_MODEL)
    bf = ml_dtypes.bfloat16
    xq = np.ascontiguousarray(q.T).astype(bf)
    xk = np.ascontiguousarray(k.T).astype(bf)
    xv = np.ascontiguousarray(v.T).astype(bf)

    Wq = np.asarray(inputs["Wq"], dtype=np.float32)
    Wk = np.asarray(inputs["Wk"], dtype=np.float32)
    Wv = np.asarray(inputs["Wv"], dtype=np.float32)
    Wo = np.asarray(inputs["Wo"], dtype=np.float32)
    bq = np.asarray(inputs["bq"], dtype=np.float32)
    bk = np.asarray(inputs["bk"], dtype=np.float32)
    bv = np.asarray(inputs["bv"], dtype=np.float32)
    bo = np.asarray(inputs["bo"], dtype=np.float32)

    scale = 1.0 / np.sqrt(D_K)
    actions, mask_tiles = _classify_mask(inputs["mask"])

    # exp-overflow guard for the no-max-subtract softmax (Cauchy-Schwarz bound)
    qn = q @ Wq.T + bq
    kn = k @ Wk.T + bk
    qmax = np.linalg.norm(qn.reshape(SEQ, N_HEADS, D_K), axis=-1).max()
    kmax = np.linalg.norm(kn.reshape(SEQ, N_HEADS, D_K), axis=-1).max()
    assert scale * qmax * kmax < 80.0, "score bound too large for exp without max-subtraction"

    shared = {
        "xq": xq, "xk": xk, "xv": xv,
        "woT": np.ascontiguousarray(Wo.T),
        "bo": np.ascontiguousarray(bo.reshape(KC, 128, 1)),
        "ident": np.eye(128, dtype=np.float32).astype(bf),
        "masks": mask_tiles,
    }
    in_maps = []
    for c in range(N_CORES):
        sl = slice(c * F, (c + 1) * F)
        m = dict(shared)
        m["wq"] = np.ascontiguousarray((Wq[sl] * scale).T).astype(bf)
        m["wk"] = np.ascontiguousarray(Wk[sl].T).astype(bf)
        m["wv"] = np.ascontiguousarray(Wv[sl].T).astype(bf)
        m["bq"] = np.ascontiguousarray((bq[sl] * scale).reshape(F, 1))
        m["bk"] = np.ascontiguousarray(bk[sl].reshape(F, 1))
        m["bv"] = np.ascontiguousarray(bv[sl].reshape(F, 1))
        in_maps.append(m)
    return in_maps, actions, mask_tiles


def _run(inputs, trace=False, trace_cores=None):
    in_maps, actions, mask_tiles = _prep(inputs)
    key = (actions.tobytes(), len(mask_tiles))
    if key not in _nc_cache:
        _nc_cache[key] = _build_nc(key[0], key[1])
    nc = _nc_cache[key]
    res = run_bass_kernel_spmd(nc, in_maps, list(range(N_CORES)),
                               trace=trace, trace_cores=trace_cores)
    blk = SEQ // N_CORES
    out = np.empty((SEQ, D_MODEL), dtype=np.float32)
    for c in range(N_CORES):
        out[c * blk:(c + 1) * blk] = res.results[c]["out_t"].T
    return out.reshape(B, S, D_MODEL), res


def kernel(**inputs) -> np.ndarray:
    out, _ = _run(inputs)
    return out
